# revision 25
# baseline (speedup 1.0000x reference)
"""Trainium2 Bass kernel for nn_Pooling_Layer (GNN message-passing pooling):
parity-split zero-waste gather + TensorEngine pooling + fused projection.

Math (per batch b): x = in_pc_pad[b] @ weight_res.T; w = |pn|*mask
normalized; out[b,p] = sum_m w[p,m] * x[id[p,m]].  We pool first in
C_IN=64 space, then project; normalization is folded into the final
PSUM->SBUF scale-copy.  Points are sharded across 8 cores; each core
handles all batches for its 1250 points.  Tables are batch-interleaved
bf16 rows (64ch x 8b = 1KB), split into separate contiguous even-id and
odd-id tables so SWDGE int16 indices (id >> 1 <= 20000) reach every row
with zero gather waste.

Each 128-point tile's 4096 slots are partitioned by neighbor-id parity
and gathered from the matching table (1KB contiguous descriptors, calls
striped across the 4 SWDGE queues in lane order).  Streams are padded to
a uniform window count across cores so the program stays SPMD.

The slot->point mapping becomes data-dependent, so the block-diagonal
lhsT is replaced by per-window weight matrices W_w[s, p] =
|pn|*mask * (p == point_of_slot), built on-device with one fused DVE
tensor_scalar (op0=is_equal against a host iota plane, op1=mult by the
weight value).  Streams are padded (weight 0, idx = pad pair) to a
uniform per-call/window count across all cores so the program stays SPMD.
"""

import numpy as np
import ml_dtypes

import concourse.bass as bass
import concourse.mybir as mybir
import concourse.tile as tile
from concourse import bacc, library_config
from concourse.bass_utils import run_bass_kernel_spmd

F32 = mybir.dt.float32
BF16 = mybir.dt.bfloat16
I16 = mybir.dt.int16

MAXN = 32
CIN = 64
COUT = 128
B = 8
NPAIRS = 20001
OVROWS = 2048            # per-core overflow rows appended to each table
NROWS_T = NPAIRS + OVROWS
EW = B * CIN             # 512 elements (1KB bf16) per gathered row
PEW = 2 * EW             # pair-row width in the table
PTS = 10000


class Params:
    def __init__(self, pts=PTS, n_cores=8, new=16, now=16):
        self.pts = pts
        self.n_cores = n_cores
        self.cpts = pts // n_cores
        self.ntl = (self.cpts + 127) // 128
        self.cpts_pad = self.ntl * 128
        self.new = new            # even windows per tile (uniform, padded)
        self.now = now            # odd windows per tile
        self.nw = new + now       # total windows per tile


def _calls(nwin):
    """Split nwin 128-slot windows into gather calls of <= 8 windows."""
    out = []
    w = 0
    while w < nwin:
        n = min(8, nwin - w)
        out.append((w, n))
        w += n
    return out


def build_nc(p: Params):
    nc = bacc.Bacc(
        "TRN2",
        target_bir_lowering=False,
        debug=False,
        num_devices=p.n_cores,
        num_swdge_queues=4,
    )
    NTL, NW = p.ntl, p.nw
    ecalls, ocalls = _calls(p.new), _calls(p.now)
    ncall_t = len(ecalls) + len(ocalls)
    idx_cols = NW * 8          # idx words per tile (NW*128/16)

    xiE = nc.dram_tensor("xiE", [NROWS_T, EW], BF16, kind="ExternalInput")
    xiO = nc.dram_tensor("xiO", [NROWS_T, EW], BF16, kind="ExternalInput")
    idxw = nc.dram_tensor("idxw", [128, NTL * idx_cols], I16, kind="ExternalInput")
    colP = nc.dram_tensor("colP", [128, NTL * NW], BF16, kind="ExternalInput")
    pnP = nc.dram_tensor("pnP", [128, NTL * NW], F32, kind="ExternalInput")
    maskP = nc.dram_tensor("maskP", [128, NTL * NW], F32, kind="ExternalInput")
    pnN = nc.dram_tensor("pnN", [p.cpts_pad, MAXN], F32, kind="ExternalInput")
    maskN = nc.dram_tensor("maskN", [p.cpts_pad, MAXN], F32, kind="ExternalInput")
    wres = nc.dram_tensor("wres", [COUT, CIN], F32, kind="ExternalInput")
    ident = nc.dram_tensor("ident", [128, 128], F32, kind="ExternalInput")
    iota = nc.dram_tensor("iota", [128, 128], BF16, kind="ExternalInput")
    out = nc.dram_tensor("out", [B * p.cpts_pad, COUT], BF16, kind="ExternalOutput")

    with tile.TileContext(nc) as tc:
        with (
            tc.tile_pool(name="const", bufs=1) as constp,
            tc.tile_pool(name="prep", bufs=1) as prep,
            tc.tile_pool(name="gather", bufs=2) as gp,
            tc.tile_pool(name="wmat", bufs=2) as wm,
            tc.tile_pool(name="work", bufs=2) as wk,
            tc.tile_pool(name="psP", bufs=2, space="PSUM") as psP,
            tc.tile_pool(name="psT", bufs=2, space="PSUM") as psT,
            tc.tile_pool(name="psO", bufs=2, space="PSUM") as psO,
        ):
            nc.gpsimd.load_library(library_config.mlp)

            # ---- constants ----
            identity = constp.tile([128, 128], F32)
            nc.sync.dma_start(out=identity[:], in_=ident[:])
            iotaP = constp.tile([128, 128], BF16)
            nc.sync.dma_start(out=iotaP[:], in_=iota[:])
            wres_sb = constp.tile([COUT, CIN], F32)
            nc.sync.dma_start(out=wres_sb[:], in_=wres[:])
            psw = psT.tile([CIN, COUT], F32, tag="psTt")
            nc.tensor.transpose(out=psw[:], in_=wres_sb[:], identity=identity[:])
            wresTb = constp.tile([128, COUT], BF16)
            nc.vector.tensor_copy(out=wresTb[0:CIN, :], in_=psw[:])
            nc.vector.tensor_copy(out=wresTb[CIN : 2 * CIN, :], in_=psw[:])

            idx_sb = constp.tile([128, NTL * idx_cols], I16)
            nc.sync.dma_start(out=idx_sb[:], in_=idxw[:])

            # ---- per-point reciprocal denominators: recip[p, t] ----
            prodN = prep.tile([128, NTL * MAXN], F32)
            nc.sync.dma_start(
                out=prodN[:].rearrange("p (t m) -> p t m", m=MAXN),
                in_=pnN[:].rearrange("(t p) m -> p t m", p=128),
            )
            maskN_sb = prep.tile([128, NTL * MAXN], F32)
            nc.sync.dma_start(
                out=maskN_sb[:].rearrange("p (t m) -> p t m", m=MAXN),
                in_=maskN[:].rearrange("(t p) m -> p t m", p=128),
            )
            nc.vector.tensor_tensor(
                out=prodN[:], in0=prodN[:], in1=maskN_sb[:], op=mybir.AluOpType.mult
            )
            denom = constp.tile([128, NTL], F32)
            nc.vector.tensor_reduce(
                out=denom[:],
                in_=prodN[:].rearrange("p (t m) -> p t m", m=MAXN),
                op=mybir.AluOpType.add,
                axis=mybir.AxisListType.X,
                apply_absolute_value=True,
            )
            nc.vector.tensor_scalar_add(denom[:], denom[:], 1e-8)
            recip = constp.tile([128, NTL], F32)
            nc.vector.reciprocal(out=recip[:], in_=denom[:])

            # ---- per-slot weight values |pnP|*maskP (permuted layout) ----
            pnP_sb = prep.tile([128, NTL * NW], F32)
            maskP_sb = prep.tile([128, NTL * NW], F32)
            colP_sb = prep.tile([128, NTL * NW], BF16)
            nc.sync.dma_start(out=pnP_sb[:], in_=pnP[:])
            nc.sync.dma_start(out=maskP_sb[:], in_=maskP[:])
            nc.sync.dma_start(out=colP_sb[:], in_=colP[:])
            wvPf = prep.tile([128, NTL * NW], F32)
            nc.scalar.activation(
                out=wvPf[:], in_=pnP_sb[:], func=mybir.ActivationFunctionType.Abs
            )
            wvP = prep.tile([128, NTL * NW], BF16)
            nc.vector.tensor_tensor(
                out=wvP[:], in0=wvPf[:], in1=maskP_sb[:], op=mybir.AluOpType.mult
            )


            # ---- main loop over 128-point tiles ----
            gcall = 0
            for t in range(NTL):
                # gather: evens then odds, into one (128, NW*512) tile
                g = gp.tile([128, NW * EW], BF16, tag="g")
                cidx = 0
                for view, calls, wbase in ((xiE[:], ecalls, 0), (xiO[:], ocalls, p.new)):
                    for (w0, nwn) in calls:
                        nidx = nwn * 128
                        col0 = t * idx_cols + cidx
                        nc.gpsimd.dma_gather(
                            g[
                                :, (wbase + w0) * EW : (wbase + w0 + nwn) * EW
                            ].rearrange("p (v e) -> p v e", e=EW),
                            view,
                            idx_sb[:, col0 : col0 + nidx // 16],
                            nidx,
                            nidx,
                            EW,
                            queue_num=gcall % 4,
                        )
                        cidx += nidx // 16
                        gcall += 1

                # build ALL W matrices for the tile with two broadcast
                # tensor_tensor ops: W[s, w, p] = (iota[s,p]==col[s,w])*wv[s,w]
                wmat = wm.tile([128, NW * 128], BF16, tag="wmat")
                wmv = wmat[:].rearrange("p (w c) -> p w c", c=128)
                iview = (
                    iotaP[:]
                    .rearrange("p (o c) -> p o c", o=1)
                    .to_broadcast([128, NW, 128])
                )
                cview = (
                    colP_sb[:, t * NW : (t + 1) * NW]
                    .rearrange("p (w o) -> p w o", o=1)
                    .to_broadcast([128, NW, 128])
                )
                wview = (
                    wvP[:, t * NW : (t + 1) * NW]
                    .rearrange("p (w o) -> p w o", o=1)
                    .to_broadcast([128, NW, 128])
                )
                nc.vector.tensor_tensor(
                    out=wmv, in0=iview, in1=cview, op=mybir.AluOpType.is_equal
                )
                nc.vector.tensor_tensor(
                    out=wmv, in0=wmv, in1=wview, op=mybir.AluOpType.mult
                )
                ps = psP.tile([128, EW], F32, tag="ps")
                for w in range(NW):
                    nc.tensor.matmul(
                        out=ps[:],
                        lhsT=wmat[:, w * 128 : (w + 1) * 128],
                        rhs=g[:, w * EW : (w + 1) * EW],
                        start=(w == 0),
                        stop=(w == NW - 1),
                    )
                pooled = wk.tile([128, EW], F32, tag="pooled")
                nc.scalar.copy(out=pooled[:], in_=ps[:])

                # 4 transposes back-to-back, Act casts, projections, stores
                psts = []
                for k in range(4):
                    pst = psT.tile([128, 128], F32, tag="psTt")
                    nc.tensor.transpose(
                        out=pst[:],
                        in_=pooled[:, k * 128 : (k + 1) * 128],
                        identity=identity[:],
                    )
                    psts.append(pst)
                poolTb = wk.tile([128, 512], BF16, tag="poolTb")
                for k in range(4):
                    nc.scalar.copy(
                        out=poolTb[:, k * 128 : (k + 1) * 128], in_=psts[k][:]
                    )
                for b in range(8):
                    k, h = b // 2, b % 2
                    pso = psO.tile([128, COUT], F32, tag="psO")
                    nc.tensor.matmul(
                        out=pso[:],
                        lhsT=poolTb[64 * h : 64 * h + 64, k * 128 : (k + 1) * 128],
                        rhs=wresTb[64 * h : 64 * h + 64, :],
                        start=True,
                        stop=True,
                    )
                    outP = wk.tile([128, COUT], BF16, tag=f"outP{b % 2}")
                    nc.scalar.activation(
                        out=outP[:],
                        in_=pso[:],
                        func=mybir.ActivationFunctionType.Copy,
                        scale=recip[:, t : t + 1],
                    )
                    r0 = b * p.cpts_pad + t * 128
                    nc.sync.dma_start(out=out[r0 : r0 + 128, :], in_=outP[:])
    nc.compile()
    return nc


def host_prep(p, in_pc_pad, ids, mask, pn, wres):
    """Returns (params, in_maps): window counts are data-dependent."""
    ids = np.asarray(ids).astype(np.int64)
    pn = np.asarray(pn, dtype=np.float32)
    mask = np.asarray(mask, dtype=np.float32)
    wres = np.asarray(wres, dtype=np.float32)
    x = np.asarray(in_pc_pad, dtype=np.float32)

    xp = np.concatenate([x, np.zeros((B, 1, CIN), np.float32)], axis=1)
    xflat = xp.transpose(1, 0, 2).reshape(2 * NPAIRS, EW)
    xiE = np.ascontiguousarray(xflat[0::2]).astype(ml_dtypes.bfloat16)
    xiO = np.ascontiguousarray(xflat[1::2]).astype(ml_dtypes.bfloat16)
    ident = np.eye(128, dtype=np.float32)
    iota = np.tile(np.arange(128, dtype=np.float32), (128, 1)).astype(ml_dtypes.bfloat16)

    # ---- per (core, tile): build parity-split slot streams ----
    n_cores = p.n_cores
    cores = []
    new_max = now_max = 0
    for c in range(n_cores):
        lo = c * p.cpts

        def pad_pts(a, dtype):
            o = np.zeros((p.cpts_pad, MAXN), dtype=dtype)
            o[: p.cpts] = a[lo : lo + p.cpts]
            return o

        ids_c = pad_pts(ids, np.int64)
        ids_c[p.cpts :] = 2 * (NPAIRS - 1)
        pn_c = pad_pts(pn, np.float32)
        mask_c = pad_pts(mask, np.float32)
        mask_c[p.cpts :] = 0          # pad points contribute zero weight
        plocal = np.repeat(np.arange(128), MAXN).reshape(128, MAXN)
        tiles = []
        # per-core overflow regions appended to the OTHER parity's table:
        # relocated even rows go into xiO2's tail, odd rows into xiE2's tail
        ovE = []   # odd rows appended to xiE (gathered via the E stream)
        ovO = []   # even rows appended to xiO (gathered via the O stream)
        for t in range(p.ntl):
            pts = slice(t * 128, (t + 1) * 128)
            idt = ids_c[pts]                     # (128, 32)
            par = (idt & 1).astype(bool)
            real = ((np.arange(128) + t * 128) < p.cpts)[:, None]

            # boolean indexing flattens row-major = point-major, m-minor
            def stream(sel):
                return [
                    (idt[sel] >> 1).astype(np.int64),
                    plocal[sel].astype(np.float32),
                    pn_c[pts][sel],
                    mask_c[pts][sel],
                ]

            e = stream(~par & real)
            o = stream(par & real)
            # rebalance: move the majority parity's overflow (beyond 2048
            # slots) into the other stream; its rows are appended to the
            # other table so the other-table idx can reach them
            if len(e[0]) > 2048:
                src_s, dst_s, ov, base = e, o, ovO, NPAIRS + len(ovO)
            else:
                src_s, dst_s, ov, base = o, e, ovE, NPAIRS + len(ovE)
            nmove = max(0, len(src_s[0]) - 2048)
            assert len(dst_s[0]) + nmove <= 2048
            if nmove:
                moved_rows = src_s[0][2048:]     # table-row ids (id>>1)
                ov.extend(moved_rows.tolist())
                for j in range(4):
                    tail = src_s[j][2048:]
                    if j == 0:
                        tail = base + np.arange(nmove, dtype=np.int64)
                    dst_s[j] = np.concatenate([dst_s[j], tail])
                    src_s[j] = src_s[j][:2048]
            tiles.append((*[a.astype(np.int16) if i == 0 else a
                            for s in (e, o) for i, a in enumerate(s)],))
        assert len(ovE) <= OVROWS and len(ovO) <= OVROWS, (len(ovE), len(ovO))
        cores.append((tiles, pn_c, mask_c, np.array(ovE, np.int64),
                      np.array(ovO, np.int64)))
        new_max = now_max = 16

    p2 = Params(pts=p.pts, n_cores=n_cores, new=new_max, now=now_max)
    NW = p2.nw
    idx_cols = NW * 8
    ecalls, ocalls = _calls(p2.new), _calls(p2.now)

    in_maps = []
    for c in range(n_cores):
        tiles, pn_c, mask_c, ovE, ovO = cores[c]
        xiE2 = np.zeros((NROWS_T, EW), ml_dtypes.bfloat16)
        xiE2[:NPAIRS] = xiE
        if len(ovE):
            xiE2[NPAIRS : NPAIRS + len(ovE)] = xiO[ovE]   # odd rows
        xiO2 = np.zeros((NROWS_T, EW), ml_dtypes.bfloat16)
        xiO2[:NPAIRS] = xiO
        if len(ovO):
            xiO2[NPAIRS : NPAIRS + len(ovO)] = xiE[ovO]   # even rows
        idx_w = np.zeros((128, p2.ntl * idx_cols), np.int16)
        colP = np.zeros((128, p2.ntl * NW), ml_dtypes.bfloat16)
        pnP = np.zeros((128, p2.ntl * NW), np.float32)
        maskP = np.zeros((128, p2.ntl * NW), np.float32)
        for t in range(p2.ntl):
            eidx, ecol, epn, emask, oidx, ocol, opn, omask = tiles[t]
            for (sidx, scol, spn, smask, nwn, wbase, calls) in (
                (eidx, ecol, epn, emask, p2.new, 0, ecalls),
                (oidx, ocol, opn, omask, p2.now, p2.new, ocalls),
            ):
                L = nwn * 128
                fi = np.full(L, NPAIRS - 1, np.int16)
                fc = np.zeros(L, np.float32)
                fp = np.zeros(L, np.float32)
                fm = np.zeros(L, np.float32)
                fi[: len(sidx)] = sidx
                fc[: len(sidx)] = scol
                fp[: len(sidx)] = spn
                fm[: len(sidx)] = smask
                # per-window planes
                wslice = slice(t * NW + wbase, t * NW + wbase + nwn)
                colP[:, wslice] = fc.reshape(nwn, 128).T
                pnP[:, wslice] = fp.reshape(nwn, 128).T
                maskP[:, wslice] = fm.reshape(nwn, 128).T
                # wrapped idx per call
                cidx = wbase * 8
                for (w0, ncw) in calls:
                    nidx = ncw * 128
                    blk = fi[w0 * 128 : w0 * 128 + nidx].reshape(nidx // 16, 16).T
                    col0 = t * idx_cols + cidx
                    idx_w[:, col0 : col0 + nidx // 16] = np.tile(blk, (8, 1))
                    cidx += nidx // 16
        in_maps.append(
            {
                "xiE": xiE2,
                "xiO": xiO2,
                "idxw": idx_w,
                "colP": colP,
                "pnP": pnP,
                "maskP": maskP,
                "pnN": pn_c,
                "maskN": mask_c,
                "wres": wres,
                "ident": ident,
                "iota": iota,
            }
        )
    return p2, in_maps


def assemble(p: Params, results):
    out = np.empty((B, p.pts, COUT), np.float32)
    for c in range(p.n_cores):
        got = np.asarray(results[c]["out"], dtype=np.float32).reshape(
            B, p.cpts_pad, COUT
        )
        out[:, c * p.cpts : (c + 1) * p.cpts, :] = got[:, : p.cpts, :]
    return out


_NC_CACHE = {}


def get_nc(p: Params):
    key = (p.pts, p.n_cores, p.new, p.now)
    if key not in _NC_CACHE:
        _NC_CACHE[key] = build_nc(p)
    return _NC_CACHE[key]


def kernel(in_pc_pad, neighbor_id_lstlst, neighbor_mask_lst, p_neighbors, weight_res):
    in_pc_pad = np.asarray(in_pc_pad)
    p0 = Params(pts=PTS, n_cores=in_pc_pad.shape[0])
    p, in_maps = host_prep(
        p0, in_pc_pad, neighbor_id_lstlst, neighbor_mask_lst, p_neighbors, weight_res
    )
    nc = get_nc(p)
    res = run_bass_kernel_spmd(nc, in_maps, core_ids=list(range(p.n_cores)))
    return assemble(p, res.results)
